# revision 10
# baseline (speedup 1.0000x reference)
"""Chamfer loss kernel for 8 Trainium2 NeuronCores.

Problem: ground_truth [4, 8192, 3], reconstruction [4, 8192, 3] (fp32).
  P[b,n,m] = ||x_n||^2 + ||y_m||^2 - 2 x_n.y_m
  loss = (mean(clamp(min_n P)) + mean(clamp(min_m P))) * 1000

Sharding: 8 independent (direction, batch) units -> 1 per core.
  cores 0..3: a = ground_truth[b],  b = reconstruction[b]   (loss_2: min over m)
  cores 4..7: a = reconstruction[b], b = ground_truth[b]    (loss_1: min over n)
Each core returns per-partition partial sums of clamp(min_b dist^2(a_i, b))
over its 8192 a-points; the host sums and combines.

Per-core kernel (v3):
  - Points are quantized to fp16 on device; all distance terms are then
    computed from the quantized values, so the kernel evaluates EXACT
    squared distances of the quantized cloud (fp16 products are exact in
    fp32). Quantization perturbs the loss by ~2e-5 relative.
  - min_b(xx + yy - 2xy) = xx + min_b(yy - 2xy): the matmul computes
    P' = yy - 2xy with K=5 fp16 operands:
      lhsT rows [x0, x1, x2, 1, 1]
      rhs  rows [-2y0, -2y1, -2y2, yy_hi, yy_lo]
    yy is split into two fp16 values (hi + lo) to keep fp32-grade
    accuracy through the fp16 operand path. fp16 streams 1 col/cycle
    (4x the fp32 rate); rows are additionally replicated at partition
    bases 0/32/64/96 and 4 matmuls run concurrently via tile_position
    row groups, which keeps TensorE off the critical path even though
    the bursty duty cycle leaves the PE HAM-throttled at 1.2 GHz.
  - Reduction: per a-tile, 8 supertiles [128, 1024] fp32 (2 PSUM banks,
    4 rolling buffers). Each supertile is consumed by one VectorE
    tensor_scalar(mult 1.0) whose accum_out (reduce op1=min) yields the
    per-chunk min [128,1] in a single pass; `csplit` of the 8 chunks are
    first copied PSUM->SBUF by ScalarE so their ts-accum runs in 2x_2P
    mode (2 elem/lane/cycle), balancing the two engines.  This beats the
    scan formulation because DVE PSUM reads stream at ~2.3 cycles/elem
    on TRN2 (the documented SBUF-latency errata) while SBUF-source
    tensor_scalar holds its spec rate.
  - finalize per a-tile: merge the 8 chunk mins, + xx, clamp at 1e-10;
    final free-axis sum -> [128, 1] partial output, combined on host.
"""

import sys

if "/opt/trn_rl_repo" not in sys.path:
    sys.path.insert(0, "/opt/trn_rl_repo")

from contextlib import ExitStack

import numpy as np

N = 8192
D = 3
P = 128
NT = N // P  # 64 a-tiles
CH = 512  # transpose/prep group width and matmul free dim
NG = N // CH  # 16 prep groups
STW = 1024  # supertile width (2 PSUM banks)

TRACE = False  # set True from test harness to capture an NTFF profile
LAST_RESULTS = None  # BassKernelResults of the most recent run (when traced)

_CACHE = {}


def _build_nc(nt_main=NT, skip_dma=False, variant="v3", reps=1, stw=STW, csplit=3,
              pack=True):
    import concourse.bacc as bacc
    import concourse.tile as tile
    from concourse import mybir
    from concourse.masks import make_identity

    f32 = mybir.dt.float32
    f16 = mybir.dt.float16

    nc = bacc.Bacc("TRN2", target_bir_lowering=False, debug=False)

    a_dram = nc.dram_tensor("a_pts", [N, D], f32, kind="ExternalInput")
    b_dram = nc.dram_tensor("b_pts", [N, D], f32, kind="ExternalInput")
    out_dram = nc.dram_tensor("partial", [P, 1], f32, kind="ExternalOutput")

    with tile.TileContext(nc) as tc, ExitStack() as ctx:
        consts = ctx.enter_context(tc.tile_pool(name="consts", bufs=1))
        sb = ctx.enter_context(tc.tile_pool(name="sb", bufs=1))
        small = ctx.enter_context(tc.tile_pool(name="small", bufs=2))
        cp_pool = ctx.enter_context(tc.tile_pool(name="cp", bufs=3))
        dst_pool = ctx.enter_context(tc.tile_pool(name="dst", bufs=3))
        prep_ctx = ExitStack()
        prep_ps = prep_ctx.enter_context(
            tc.tile_pool(name="prep_ps", bufs=2, space="PSUM")
        )

        ident = consts.tile([P, P], f32)
        make_identity(nc, ident)

        # natural-layout fp32 staging: [128 points-in-tile, 64 tiles, 3]
        astage = sb.tile([P, NT, D], f32)
        bstage = sb.tile([P, NT, D], f32)
        if skip_dma:
            nc.vector.memset(astage, 0.5)
            nc.vector.memset(bstage, 0.25)
        else:
            nc.sync.dma_start(
                out=astage, in_=a_dram.ap().rearrange("(t p) d -> p t d", p=P)
            )
            nc.sync.dma_start(
                out=bstage, in_=b_dram.ap().rearrange("(t p) d -> p t d", p=P)
            )

        # quantize both clouds to fp16; all downstream math uses the
        # quantized values so distances are exact-in-fp32 of the
        # quantized points
        aq16 = sb.tile([P, NT, D], f16)
        nc.vector.tensor_copy(aq16, astage)
        bq16 = sb.tile([P, NT, D], f16)
        nc.vector.tensor_copy(bq16, bstage)

        # upcast staging of quantized values (transposed on the PE in
        # fp32, downcast again at the PSUM->SBUF copy: exact round trip)
        aqs = sb.tile([P, NT, D], f32)
        nc.vector.tensor_copy(aqs, aq16)
        bq32 = sb.tile([P, NT, D], f32)
        nc.vector.tensor_copy(bq32, bq16)

        # xx per a-point (from quantized coords), natural layout [128, 64]
        sqa = sb.tile([P, NT, D], f32)
        nc.vector.tensor_mul(sqa, aqs, aqs)
        xx = sb.tile([P, NT], f32)
        nc.vector.tensor_reduce(
            out=xx, in_=sqa, axis=mybir.AxisListType.X, op=mybir.AluOpType.add
        )

        # b-side staging [128, 64, 5]: cols 0:3 = -2*y_q, col 3 = yy_hi,
        # col 4 = yy_lo (both stored as upcast-exact fp16 values)
        bqs = sb.tile([P, NT, 5], f32)
        nc.vector.tensor_scalar(
            out=bqs[:, :, 0:D],
            in0=bq32,
            scalar1=-2.0,
            scalar2=None,
            op0=mybir.AluOpType.mult,
        )
        sqb = sb.tile([P, NT, D], f32)
        nc.vector.tensor_mul(sqb, bq32, bq32)
        yy = sb.tile([P, NT, 1], f32)
        nc.vector.tensor_reduce(
            out=yy, in_=sqb, axis=mybir.AxisListType.X, op=mybir.AluOpType.add
        )
        yyh16 = sb.tile([P, NT, 1], f16)
        nc.vector.tensor_copy(yyh16, yy)
        nc.vector.tensor_copy(bqs[:, :, 3:4], yyh16)  # upcast yy_hi
        resid = sb.tile([P, NT, 1], f32)
        nc.vector.tensor_tensor(
            out=resid, in0=yy, in1=bqs[:, :, 3:4], op=mybir.AluOpType.subtract
        )
        yyl16 = sb.tile([P, NT, 1], f16)
        nc.vector.tensor_copy(yyl16, resid)
        nc.vector.tensor_copy(bqs[:, :, 4:5], yyl16)  # upcast yy_lo

        # K-major fp16 operands: LHS rows [x0,x1,x2,1,1], RHS rows
        # [-2y0,-2y1,-2y2,yy_hi,yy_lo].  When packing, rows are replicated
        # at partition bases 32/64/96 for tile_position row-group packing.
        LHS = sb.tile([P if pack else 5, N], f16)
        RHS = sb.tile([P if pack else 5, N], f16)
        # rows 3:5 stay 1.0 (the ones rows); rows 0:3 are overwritten by the
        # per-group transposed-coordinate copies below
        nc.vector.memset(LHS[0:5, :], 1.0)
        for g in range(NG):
            tpa = prep_ps.tile([D, CH], f32, tag="tpa")
            for c in range(4):
                t = 4 * g + c
                nc.tensor.transpose(tpa[:, c * P : (c + 1) * P], aqs[:, t, :], ident)
            nc.scalar.copy(LHS[0:D, g * CH : (g + 1) * CH], tpa)  # downcast
            tpb = prep_ps.tile([5, CH], f32, tag="tpb")
            for c in range(4):
                t = 4 * g + c
                nc.tensor.transpose(tpb[:, c * P : (c + 1) * P], bqs[:, t, :], ident)
            nc.scalar.copy(RHS[0:5, g * CH : (g + 1) * CH], tpb)  # downcast
        if pack:
            for r in (32, 64, 96):
                nc.sync.dma_start(out=LHS[r : r + 5, :], in_=LHS[0:5, :])
                nc.sync.dma_start(out=RHS[r : r + 5, :], in_=RHS[0:5, :])

        prep_ctx.close()
        psum_bufs = (2 * 1024) // stw * 2  # 4 bufs at stw=1024
        main_ps = ctx.enter_context(
            tc.tile_pool(name="main_ps", bufs=psum_bufs, space="PSUM")
        )

        res = sb.tile([P, NT], f32)
        if nt_main < NT:
            nc.vector.memset(res, 0.0)
        junk = None
        if variant == "scanonly":
            junk = sb.tile([P, stw], f32)
            nc.vector.memset(junk, 1.0e30)
        trash = None
        if variant in ("v3", "tsonly"):
            # write target for the ts-accum ops; the data is dead, only the
            # accum_out matters.  bf16 halves the write traffic.
            trash = sb.tile([P, stw], mybir.dt.bfloat16)

        rep_ctx = ExitStack()
        if reps > 1:  # timing amplification: re-execute the main loop
            rep_ctx.enter_context(tc.For_i(0, reps, 1))

        mm_per_st = stw // CH
        nch = N // stw  # supertile chunks per a-tile

        def emit_mms(pb, t, chunk_idx):
            # fill supertile `pb` with P' for a-tile t, b-range
            # [chunk_idx*stw, (chunk_idx+1)*stw)
            for h in range(mm_per_st):
                col0 = chunk_idx * stw + h * CH
                if pack:
                    r = (chunk_idx * mm_per_st + h) % 4
                    nc.tensor.matmul(
                        pb[:, h * CH : (h + 1) * CH],
                        LHS[32 * r : 32 * r + 5, t * P : (t + 1) * P],
                        RHS[32 * r : 32 * r + 5, col0 : col0 + CH],
                        start=True,
                        stop=True,
                        tile_position=(32 * r, 0),
                    )
                else:
                    nc.tensor.matmul(
                        pb[:, h * CH : (h + 1) * CH],
                        LHS[0:5, t * P : (t + 1) * P],
                        RHS[0:5, col0 : col0 + CH],
                        start=True,
                        stop=True,
                    )

        if variant in ("v3", "tsonly"):
            # spread the ScalarE-copied chunks evenly through the tile
            csp = 0 if variant == "tsonly" else csplit
            copied = {int((i + 0.5) * nch / csp) for i in range(csp)} if csp else set()
            for t in range(nt_main):
                pm = small.tile([P, nch], f32, tag="pm")
                for k in range(nch):
                    pb = main_ps.tile([P, stw], f32, tag="st")
                    emit_mms(pb, t, k)
                    if k in copied:
                        cp = cp_pool.tile([P, stw], f32, tag="cp")
                        nc.scalar.copy(cp, pb)
                        src = cp
                    else:
                        src = pb
                    nc.vector.tensor_scalar(
                        out=trash,
                        in0=src,
                        scalar1=1.0,
                        scalar2=None,
                        op0=mybir.AluOpType.mult,
                        op1=mybir.AluOpType.min,
                        accum_out=pm[:, k : k + 1],
                    )
                mmin = small.tile([P, 1], f32, tag="mmin")
                nc.vector.tensor_reduce(
                    out=mmin, in_=pm, axis=mybir.AxisListType.X,
                    op=mybir.AluOpType.min,
                )
                nc.vector.tensor_scalar(
                    out=res[:, t : t + 1],
                    in0=mmin,
                    scalar1=xx[:, t : t + 1],
                    scalar2=1e-10,
                    op0=mybir.AluOpType.add,
                    op1=mybir.AluOpType.max,
                )
        else:
            npair = nch // 2
            for t in range(nt_main):
                prev_init = None  # AP of the running min ([P,1]) or None
                for k in range(npair):
                    pb0 = main_ps.tile([P, stw], f32, tag="st")
                    pb1 = main_ps.tile([P, stw], f32, tag="st")
                    emit_mms(pb0, t, 2 * k)
                    emit_mms(pb1, t, 2 * k + 1)
                    if variant == "mmonly":
                        continue
                    if variant == "scanonly":
                        dst = dst_pool.tile([P, stw], f32, tag="dst")
                        nc.vector.tensor_tensor_scan(
                            out=dst,
                            data0=pb0,
                            initial=(1.0e30 if prev_init is None else prev_init),
                            data1=junk,
                            op0=mybir.AluOpType.min,
                            op1=mybir.AluOpType.min,
                        )
                        prev_init = dst[:, stw - 1 : stw]
                        continue
                    cp = cp_pool.tile([P, stw], f32, tag="cp")
                    nc.scalar.copy(cp, pb1)
                    if variant == "copyonly":
                        prev_init = cp[:, 0:1]
                        continue
                    dst = dst_pool.tile([P, stw], f32, tag="dst")
                    nc.vector.tensor_tensor_scan(
                        out=dst,
                        data0=pb0,
                        initial=(1.0e30 if variant == "nochain" or prev_init is None
                                 else prev_init),
                        data1=cp,
                        op0=mybir.AluOpType.min,
                        op1=mybir.AluOpType.min,
                    )
                    prev_init = dst[:, stw - 1 : stw]
                if variant == "mmonly":
                    continue
                # res[:, t] = max(min + xx[:, t], 1e-10)
                nc.vector.tensor_scalar(
                    out=res[:, t : t + 1],
                    in0=prev_init,
                    scalar1=xx[:, t : t + 1],
                    scalar2=1e-10,
                    op0=mybir.AluOpType.add,
                    op1=mybir.AluOpType.max,
                )
            if variant == "mmonly":
                nc.vector.memset(res, 7.0)

        rep_ctx.close()

        res1 = small.tile([P, 1], f32)
        nc.vector.tensor_reduce(
            out=res1, in_=res, axis=mybir.AxisListType.X, op=mybir.AluOpType.add
        )
        nc.sync.dma_start(out=out_dram.ap(), in_=res1)

    nc.compile()
    return nc


def _get_nc(**kw):
    key = tuple(sorted(kw.items())) or "nc"
    if key not in _CACHE:
        _CACHE[key] = _build_nc(**kw)
    return _CACHE[key]


def _run(nc, gt, rc, B):
    from concourse.bass_utils import run_bass_kernel_spmd

    in_maps = []
    for b in range(B):  # cores 0..3: min over reconstruction for each gt point
        in_maps.append({"a_pts": gt[b], "b_pts": rc[b]})
    for b in range(B):  # cores 4..7: min over gt for each reconstruction point
        in_maps.append({"a_pts": rc[b], "b_pts": gt[b]})

    try:
        results = run_bass_kernel_spmd(
            nc, in_maps, core_ids=list(range(2 * B)), trace=TRACE
        )
    except Exception:
        # transient NRT_EXEC_UNIT_UNRECOVERABLE has been observed after
        # heavy preceding runs; one retry recovers
        results = run_bass_kernel_spmd(
            nc, in_maps, core_ids=list(range(2 * B)), trace=TRACE
        )
    return results


def kernel(ground_truth: np.ndarray, reconstruction: np.ndarray) -> np.ndarray:
    global LAST_RESULTS

    gt = np.ascontiguousarray(ground_truth, dtype=np.float32)
    rc = np.ascontiguousarray(reconstruction, dtype=np.float32)
    B = gt.shape[0]
    assert gt.shape == (B, N, D) and rc.shape == (B, N, D)

    nc = _get_nc()
    results = _run(nc, gt, rc, B)
    LAST_RESULTS = results

    partials = np.array(
        [float(np.sum(r["partial"].astype(np.float64))) for r in results.results]
    )
    loss_2 = partials[:B].sum() / (B * N)
    loss_1 = partials[B:].sum() / (B * N)
    total = (loss_1 + loss_2) * 1000.0
    return np.asarray(total, dtype=np.float32)


# revision 23
# speedup vs baseline: 1.6056x; 1.6056x over previous
"""Chamfer loss kernel for 8 Trainium2 NeuronCores.

Problem: ground_truth [4, 8192, 3], reconstruction [4, 8192, 3] (fp32).
  P[b,n,m] = ||x_n||^2 + ||y_m||^2 - 2 x_n.y_m
  loss = (mean(clamp(min_n P)) + mean(clamp(min_m P))) * 1000

Sharding: 8 independent (direction, batch) units -> 1 per core.
  cores 0..3: a = ground_truth[b],  b = reconstruction[b]   (loss_2: min over m)
  cores 4..7: a = reconstruction[b], b = ground_truth[b]    (loss_1: min over n)
Each core returns per-partition partial sums of clamp(min_b dist^2(a_i, b))
over its 8192 a-points; the host sums and combines.

Per-core kernel (v3):
  - Points are quantized to fp16 on device; all distance terms are then
    computed from the quantized values, so the kernel evaluates EXACT
    squared distances of the quantized cloud (fp16 products are exact in
    fp32). Quantization perturbs the loss by ~2e-5 relative.
  - The matmul computes the full D^2 = xx + yy - 2xy with K=7 fp16
    operands (hi/lo splits keep fp32-grade accuracy through fp16):
      lhsT rows [x0, x1, x2, xx_hi, xx_lo, 1, 1]
      rhs  rows [-2y0, -2y1, -2y2, 1, 1, yy_hi, yy_lo]
    D^2 >= 0 makes bf16 intermediates safe near the min (bf16 error is
    relative). fp16 streams 1 col/cycle (4x the fp32 rate); rows are
    additionally replicated at partition bases 0/32/64/96 and 4 matmuls
    run concurrently via tile_position row groups, which keeps TensorE
    off the critical path even though the bursty duty cycle leaves the
    PE HAM-throttled at 1.2 GHz.
  - Reduction: per a-tile, 8 supertiles [128, 1024] fp32 (2 PSUM banks,
    4 rolling buffers). Each supertile is consumed by one VectorE
    tensor_scalar(mult 1.0) whose accum_out (reduce op1=min) yields the
    per-chunk min [128,1] in a single pass (~1.3 cyc/elem from PSUM —
    unlike tensor_tensor_scan, which is recurrence-limited to ~2.6);
    `csplit` of the 8 chunks are instead copied PSUM->SBUF-bf16 by
    ScalarE and min-accumulated from SBUF, offloading the DVE.  Those
    deferred ts-accums run one a-tile later so the ScalarE copy has a
    full tile of lead time and never blocks the strict-FIFO DVE queue.
  - finalize per a-tile: merge the 8 chunk mins, clamp at 1e-10; final
    free-axis sum -> [128, 1] partial output, combined on host.
"""

import sys

if "/opt/trn_rl_repo" not in sys.path:
    sys.path.insert(0, "/opt/trn_rl_repo")

from contextlib import ExitStack

import numpy as np

N = 8192
D = 3
P = 128
NT = N // P  # 64 a-tiles
CH = 512  # transpose/prep group width and matmul free dim
NG = N // CH  # 16 prep groups
STW = 1024  # supertile width (2 PSUM banks)

TRACE = False  # set True from test harness to capture an NTFF profile
LAST_RESULTS = None  # BassKernelResults of the most recent run (when traced)

_CACHE = {}


def _build_nc(nt_main=NT, skip_dma=False, variant="v3", reps=1, stw=STW, csplit=3,
              pack=True):
    import concourse.bacc as bacc
    import concourse.tile as tile
    from concourse import mybir
    from concourse.masks import make_identity

    f32 = mybir.dt.float32
    f16 = mybir.dt.float16

    nc = bacc.Bacc("TRN2", target_bir_lowering=False, debug=False)

    a_dram = nc.dram_tensor("a_pts", [N, D], f32, kind="ExternalInput")
    b_dram = nc.dram_tensor("b_pts", [N, D], f32, kind="ExternalInput")
    out_dram = nc.dram_tensor("partial", [P, 1], f32, kind="ExternalOutput")

    with tile.TileContext(nc) as tc, ExitStack() as ctx:
        consts = ctx.enter_context(tc.tile_pool(name="consts", bufs=1))
        sb = ctx.enter_context(tc.tile_pool(name="sb", bufs=1))
        small = ctx.enter_context(tc.tile_pool(name="small", bufs=3))
        cp_pool = ctx.enter_context(tc.tile_pool(name="cp", bufs=3))
        dst_pool = ctx.enter_context(tc.tile_pool(name="dst", bufs=3))
        prep_ctx = ExitStack()
        prep_ps = prep_ctx.enter_context(
            tc.tile_pool(name="prep_ps", bufs=2, space="PSUM")
        )

        ident = consts.tile([P, P], f32)
        make_identity(nc, ident)

        # natural-layout fp32 staging: [128 points-in-tile, 64 tiles, 3]
        astage = sb.tile([P, NT, D], f32)
        bstage = sb.tile([P, NT, D], f32)
        if skip_dma:
            nc.vector.memset(astage, 0.5)
            nc.vector.memset(bstage, 0.25)
        else:
            nc.sync.dma_start(
                out=astage, in_=a_dram.ap().rearrange("(t p) d -> p t d", p=P)
            )
            nc.sync.dma_start(
                out=bstage, in_=b_dram.ap().rearrange("(t p) d -> p t d", p=P)
            )

        # quantize both clouds to fp16; all downstream math uses the
        # quantized values so distances are exact-in-fp32 of the
        # quantized points
        aq16 = sb.tile([P, NT, D], f16)
        nc.vector.tensor_copy(aq16, astage)
        bq16 = sb.tile([P, NT, D], f16)
        nc.vector.tensor_copy(bq16, bstage)

        # upcast staging of quantized values (transposed on the PE in
        # fp32, downcast again at the PSUM->SBUF copy: exact round trip)
        aqs = sb.tile([P, NT, D], f32)
        nc.vector.tensor_copy(aqs, aq16)
        bq32 = sb.tile([P, NT, D], f32)
        nc.vector.tensor_copy(bq32, bq16)

        # xx per a-point (from quantized coords), natural layout [128, 64],
        # split hi/lo for the fp16 operand path (K=7 computes the full D^2
        # in PSUM so intermediates are non-negative and bf16-safe)
        sqa = sb.tile([P, NT, D], f32)
        nc.vector.tensor_mul(sqa, aqs, aqs)
        xx = sb.tile([P, NT, 1], f32)
        nc.vector.tensor_reduce(
            out=xx, in_=sqa, axis=mybir.AxisListType.X, op=mybir.AluOpType.add
        )
        # a-side staging [128, 64, 7]:
        # cols 0:3 = x_q, col 3 = xx_hi, col 4 = xx_lo, cols 5:7 = 1.0
        aqs7 = sb.tile([P, NT, 7], f32)
        nc.vector.tensor_copy(aqs7[:, :, 0:D], aqs)
        nc.vector.memset(aqs7[:, :, 5:7], 1.0)
        xxh16 = sb.tile([P, NT, 1], f16)
        nc.vector.tensor_copy(xxh16, xx)
        nc.vector.tensor_copy(aqs7[:, :, 3:4], xxh16)  # upcast xx_hi
        xresid = sb.tile([P, NT, 1], f32)
        nc.vector.tensor_tensor(
            out=xresid, in0=xx, in1=aqs7[:, :, 3:4], op=mybir.AluOpType.subtract
        )
        xxl16 = sb.tile([P, NT, 1], f16)
        nc.vector.tensor_copy(xxl16, xresid)
        nc.vector.tensor_copy(aqs7[:, :, 4:5], xxl16)  # upcast xx_lo

        # b-side staging [128, 64, 7]: cols 0:3 = -2*y_q, cols 3:5 = 1.0,
        # col 5 = yy_hi, col 6 = yy_lo (stored as upcast-exact fp16 values)
        bqs = sb.tile([P, NT, 7], f32)
        nc.vector.tensor_scalar(
            out=bqs[:, :, 0:D],
            in0=bq32,
            scalar1=-2.0,
            scalar2=None,
            op0=mybir.AluOpType.mult,
        )
        nc.vector.memset(bqs[:, :, 3:5], 1.0)
        sqb = sb.tile([P, NT, D], f32)
        nc.vector.tensor_mul(sqb, bq32, bq32)
        yy = sb.tile([P, NT, 1], f32)
        nc.vector.tensor_reduce(
            out=yy, in_=sqb, axis=mybir.AxisListType.X, op=mybir.AluOpType.add
        )
        yyh16 = sb.tile([P, NT, 1], f16)
        nc.vector.tensor_copy(yyh16, yy)
        nc.vector.tensor_copy(bqs[:, :, 5:6], yyh16)  # upcast yy_hi
        resid = sb.tile([P, NT, 1], f32)
        nc.vector.tensor_tensor(
            out=resid, in0=yy, in1=bqs[:, :, 5:6], op=mybir.AluOpType.subtract
        )
        yyl16 = sb.tile([P, NT, 1], f16)
        nc.vector.tensor_copy(yyl16, resid)
        nc.vector.tensor_copy(bqs[:, :, 6:7], yyl16)  # upcast yy_lo

        # K=7 fp16 operands: LHS rows [x0,x1,x2,xx_hi,xx_lo,1,1], RHS rows
        # [-2y0,-2y1,-2y2,1,1,yy_hi,yy_lo] -> PSUM holds the full D^2 >= 0
        # (bf16-safe near the min).  When packing, rows are replicated at
        # partition bases 32/64/96 for tile_position row-group packing.
        KK = 7
        LHS = sb.tile([P if pack else KK, N], f16)
        RHS = sb.tile([P if pack else KK, N], f16)
        for g in range(NG):
            tpa = prep_ps.tile([KK, CH], f32, tag="tpa")
            for c in range(4):
                t = 4 * g + c
                nc.tensor.transpose(tpa[:, c * P : (c + 1) * P], aqs7[:, t, :], ident)
            if g % 2 == 0:
                nc.scalar.copy(LHS[0:KK, g * CH : (g + 1) * CH], tpa)  # downcast
            else:
                nc.vector.tensor_copy(LHS[0:KK, g * CH : (g + 1) * CH], tpa)
            tpb = prep_ps.tile([KK, CH], f32, tag="tpb")
            for c in range(4):
                t = 4 * g + c
                nc.tensor.transpose(tpb[:, c * P : (c + 1) * P], bqs[:, t, :], ident)
            if g % 2 == 1:
                nc.scalar.copy(RHS[0:KK, g * CH : (g + 1) * CH], tpb)  # downcast
            else:
                nc.vector.tensor_copy(RHS[0:KK, g * CH : (g + 1) * CH], tpb)
        if pack:
            for r in (32, 64, 96):
                nc.sync.dma_start(out=LHS[r : r + KK, :], in_=LHS[0:KK, :])
                nc.sync.dma_start(out=RHS[r : r + KK, :], in_=RHS[0:KK, :])

        prep_ctx.close()
        psum_bufs = (2 * 1024) // stw * 2  # 4 bufs at stw=1024
        main_ps = ctx.enter_context(
            tc.tile_pool(name="main_ps", bufs=psum_bufs, space="PSUM")
        )

        res = sb.tile([P, NT], f32)
        if nt_main < NT:
            nc.vector.memset(res, 0.0)
        junk = None
        if variant == "scanonly":
            junk = sb.tile([P, stw], f32)
            nc.vector.memset(junk, 1.0e30)
        trash = None
        if variant in ("v3", "tsonly"):
            # write target for the ts-accum ops; the data is dead, only the
            # accum_out matters.  bf16 halves the write traffic.
            tw = max(stw, (0 if variant == "tsonly" else csplit) * stw)
            trash = sb.tile([P, tw], mybir.dt.bfloat16)

        rep_ctx = ExitStack()
        if reps > 1:  # timing amplification: re-execute the main loop
            rep_ctx.enter_context(tc.For_i(0, reps, 1))

        mm_per_st = stw // CH
        nch = N // stw  # supertile chunks per a-tile

        def emit_mms(pb, t, chunk_idx):
            # fill supertile `pb` with P' for a-tile t, b-range
            # [chunk_idx*stw, (chunk_idx+1)*stw)
            for h in range(mm_per_st):
                col0 = chunk_idx * stw + h * CH
                if pack:
                    r = (chunk_idx * mm_per_st + h) % 4
                    nc.tensor.matmul(
                        pb[:, h * CH : (h + 1) * CH],
                        LHS[32 * r : 32 * r + 7, t * P : (t + 1) * P],
                        RHS[32 * r : 32 * r + 7, col0 : col0 + CH],
                        start=True,
                        stop=True,
                        tile_position=(32 * r, 0),
                    )
                else:
                    nc.tensor.matmul(
                        pb[:, h * CH : (h + 1) * CH],
                        LHS[0:7, t * P : (t + 1) * P],
                        RHS[0:7, col0 : col0 + CH],
                        start=True,
                        stop=True,
                    )

        if variant in ("v3", "tsonly"):
            # spread the ScalarE-copied chunks evenly through the tile
            csp = 0 if variant == "tsonly" else csplit
            copied = {int((i + 0.5) * nch / csp) for i in range(csp)} if csp else set()

            def ts_accum(src, accum):
                nc.vector.tensor_scalar(
                    out=trash[:, 0 : src.free_size()],
                    in0=src,
                    scalar1=1.0,
                    scalar2=None,
                    op0=mybir.AluOpType.mult,
                    op1=mybir.AluOpType.min,
                    accum_out=accum,
                )

            def finalize(pm_prev, t_prev):
                # chunk-min merge; the 1e-10 clamp is applied once over the
                # whole res tile after the loop
                nc.vector.tensor_reduce(
                    out=res[:, t_prev : t_prev + 1],
                    in_=pm_prev,
                    axis=mybir.AxisListType.X,
                    op=mybir.AluOpType.min,
                )

            # the ts-accum of a ScalarE-copied chunk is deferred by one
            # a-tile so the copy has a full tile (~7us) of lead time and
            # never blocks the strict-FIFO DVE queue.  The csp copies of a
            # tile land in one wide bf16 tile and are min-accumulated by a
            # single deferred 4x-mode tensor_scalar.
            npm = nch - csp + 1 if csp else nch
            pending = None  # (cp_tile, pm_slot_ap) from the previous tile
            prev = None  # (pm, t) of the previous tile
            for t in range(nt_main):
                pm = small.tile([P, npm], f32, tag="pm")
                cp = None
                if csp:
                    cp = cp_pool.tile([P, csp * stw], mybir.dt.bfloat16, tag="cp")
                ci = 0
                di = 0
                for k in range(nch):
                    pb = main_ps.tile([P, stw], f32, tag="st")
                    emit_mms(pb, t, k)
                    if k in copied:
                        # downcast; D^2 is bf16-safe
                        nc.scalar.copy(cp[:, ci * stw : (ci + 1) * stw], pb)
                        ci += 1
                    else:
                        ts_accum(pb, pm[:, di : di + 1])
                        di += 1
                if prev is not None:
                    # drain the previous tile's copied chunks, finalize it
                    if pending is not None:
                        ts_accum(pending[0], pending[1])
                    finalize(prev[0], prev[1])
                if csp:
                    pending = (cp, pm[:, npm - 1 : npm])
                prev = (pm, t)
            # drain the final tile
            if pending is not None:
                ts_accum(pending[0], pending[1])
            if prev is not None:
                finalize(prev[0], prev[1])
            if nt_main:
                nc.vector.tensor_scalar(
                    out=res,
                    in0=res,
                    scalar1=1e-10,
                    scalar2=None,
                    op0=mybir.AluOpType.max,
                )
        else:
            npair = nch // 2
            for t in range(nt_main):
                prev_init = None  # AP of the running min ([P,1]) or None
                for k in range(npair):
                    pb0 = main_ps.tile([P, stw], f32, tag="st")
                    pb1 = main_ps.tile([P, stw], f32, tag="st")
                    emit_mms(pb0, t, 2 * k)
                    emit_mms(pb1, t, 2 * k + 1)
                    if variant == "mmonly":
                        continue
                    if variant == "scanonly":
                        dst = dst_pool.tile([P, stw], f32, tag="dst")
                        nc.vector.tensor_tensor_scan(
                            out=dst,
                            data0=pb0,
                            initial=(1.0e30 if prev_init is None else prev_init),
                            data1=junk,
                            op0=mybir.AluOpType.min,
                            op1=mybir.AluOpType.min,
                        )
                        prev_init = dst[:, stw - 1 : stw]
                        continue
                    cp = cp_pool.tile([P, stw], f32, tag="cp")
                    nc.scalar.copy(cp, pb1)
                    if variant == "copyonly":
                        prev_init = cp[:, 0:1]
                        continue
                    dst = dst_pool.tile([P, stw], f32, tag="dst")
                    nc.vector.tensor_tensor_scan(
                        out=dst,
                        data0=pb0,
                        initial=(1.0e30 if variant == "nochain" or prev_init is None
                                 else prev_init),
                        data1=cp,
                        op0=mybir.AluOpType.min,
                        op1=mybir.AluOpType.min,
                    )
                    prev_init = dst[:, stw - 1 : stw]
                if variant == "mmonly":
                    continue
                # res[:, t] = max(min, 1e-10): PSUM already holds D^2
                nc.vector.tensor_scalar(
                    out=res[:, t : t + 1],
                    in0=prev_init,
                    scalar1=1e-10,
                    scalar2=None,
                    op0=mybir.AluOpType.max,
                )
            if variant == "mmonly":
                nc.vector.memset(res, 7.0)

        rep_ctx.close()

        res1 = small.tile([P, 1], f32)
        nc.vector.tensor_reduce(
            out=res1, in_=res, axis=mybir.AxisListType.X, op=mybir.AluOpType.add
        )
        nc.sync.dma_start(out=out_dram.ap(), in_=res1)

    nc.compile()
    return nc


def _get_nc(**kw):
    key = tuple(sorted(kw.items())) or "nc"
    if key not in _CACHE:
        _CACHE[key] = _build_nc(**kw)
    return _CACHE[key]


def _run(nc, gt, rc, B):
    from concourse.bass_utils import run_bass_kernel_spmd

    in_maps = []
    for b in range(B):  # cores 0..3: min over reconstruction for each gt point
        in_maps.append({"a_pts": gt[b], "b_pts": rc[b]})
    for b in range(B):  # cores 4..7: min over gt for each reconstruction point
        in_maps.append({"a_pts": rc[b], "b_pts": gt[b]})

    try:
        results = run_bass_kernel_spmd(
            nc, in_maps, core_ids=list(range(2 * B)), trace=TRACE
        )
    except Exception:
        # transient NRT_EXEC_UNIT_UNRECOVERABLE has been observed after
        # heavy preceding runs; one retry recovers
        results = run_bass_kernel_spmd(
            nc, in_maps, core_ids=list(range(2 * B)), trace=TRACE
        )
    return results


def kernel(ground_truth: np.ndarray, reconstruction: np.ndarray) -> np.ndarray:
    global LAST_RESULTS

    gt = np.ascontiguousarray(ground_truth, dtype=np.float32)
    rc = np.ascontiguousarray(reconstruction, dtype=np.float32)
    B = gt.shape[0]
    assert gt.shape == (B, N, D) and rc.shape == (B, N, D)

    nc = _get_nc()
    results = _run(nc, gt, rc, B)
    LAST_RESULTS = results

    partials = np.array(
        [float(np.sum(r["partial"].astype(np.float64))) for r in results.results]
    )
    loss_2 = partials[:B].sum() / (B * N)
    loss_1 = partials[B:].sum() / (B * N)
    total = (loss_1 + loss_2) * 1000.0
    return np.asarray(total, dtype=np.float32)
